# revision 32
# baseline (speedup 1.0000x reference)
"""Trainium2 Bass kernel: paged int8-KV-cache GQA decode attention, 8-core SPMD.

Contract: kernel(**inputs) takes the FULL unsharded numpy inputs (as produced by
the reference setup_inputs) and returns the FULL [32, 4096] float32 output.

Strategy (data parallel + split-K over token windows):
  - Work units are (sequence, token-window) RANGES, flash-decoding style:
    every core runs an identical program over R ranges of compile-time tile
    counts rs[i]; a host-side solver cuts the 32 sequences' token streams
    into 8 windows per range size so that padding is ~the global remainder
    (<1%), vs ~12% for whole-sequence slotting.  Per-range partial (PV, Z)
    pairs are summed per sequence on the host (exact: no max-subtraction is
    used, so partial softmax sums add linearly).
  - K/V int8 cache values are gathered per block_tables into per-core packed
    int8 buffers (1 byte per element in HBM), laid out block-major so every
    HBM->SBUF DMA is one contiguous run per partition.  DMAs move RAW int8
    (the SDMA engine pool is bound by read+write bytes, ~464 GB/s/core; a
    casting DMA would pay 3B/elem, raw pays 2B/elem); the exact int8 -> bf16
    casts run on DVE (K + 1/4 of V) and ACT (3/4 of V) as full-tile copies,
    off the DMA fabric.
  - Work is chopped into <=BT-token-tile blocks per (range, kvh-group) and
    emitted as a staggered software pipeline (DMA 2 blocks ahead, casts 2
    blocks ahead of the score ops) so no engine FIFO head-blocks another.
  - Per block: QK matmuls (K^T tile as stationary operand) into psum cols
    0:4 -> DVE mul by k_scale*softmax_scale (in place) -> DVE add of
    ln(v_scale) into cols 4:8 -> ONE ACT exp producing [e | ev] in the PV
    lhsT layout; then the PV+Z fused matmul lhsT=[e|ev], rhs=[V|mask]
    accumulated into a per-(range,group) PSUM bank, kvh j2 on PE column-
    group j2 (tile_position).  Rows 32*j2+0..3 hold Z (col 128); rows
    32*j2+4..7 hold PV (cols 0..127).  Masked tokens cost nothing extra:
    K/V/mask are host-zeroed and ln(v_scale) is -1e30 there.
  Softmax skips max-subtraction (scores are O(20) at most; fp32 exp is safe).
"""

import math
import random
import sys
from contextlib import ExitStack

import numpy as np

sys.path.insert(0, "/opt/trn_rl_repo")

import ml_dtypes  # noqa: E402

import concourse.bass as bass  # noqa: E402
import concourse.mybir as mybir  # noqa: E402
import concourse.tile as tile  # noqa: E402
from concourse import bacc  # noqa: E402
from concourse.bass_utils import run_bass_kernel_spmd  # noqa: E402

BF16 = ml_dtypes.bfloat16

B = 32
NUM_HEADS = 32
KVH = 8
D = 128
REP = NUM_HEADS // KVH  # 4
BLOCK_SIZE = 256
T = 4096
P = 128
DV = D + 1  # V columns + mask column
SCALE = 1.0 / float(np.sqrt(D))
NCORES = 8
BT = 14     # token tiles per pipeline block


# ---------------------------------------------------------------------------
# host-side planning
# ---------------------------------------------------------------------------

def _greedy_assign(sizes, tiles):
    """Cut sequences' tile tails into 8 windows per range size (desc order).

    Returns (pad, plan) where plan[range_index] is a list of up to 8
    (seq, w0_tile, take) triples (core order), or None entries.
    """
    rem = [(int(t), b) for b, t in enumerate(tiles)]
    pad = 0
    order = np.argsort([-s for s in sizes], kind="stable")
    plan = [[None] * NCORES for _ in sizes]
    for ri in order:
        r = sizes[ri]
        for c in range(NCORES):
            rem.sort(reverse=True)
            t0, b = rem[0]
            if t0 == 0:
                pad += r
                continue
            take = min(r, t0)
            pad += r - take
            rem[0] = (t0 - take, b)
            plan[ri][c] = (b, t0 - take, take)
    left = sum(t for t, b in rem)
    return (pad if left == 0 else None), plan


def _plan(context_lens):
    """Choose shared range sizes + (core, range) -> (seq, window) assignment.

    Padding is exactly NCORES*sum(rs) - total for any feasible plan, so
    search ascending per-core totals and take the first feasible config.
    """
    tiles = [int(math.ceil(int(c) / P)) for c in context_lens]
    total = sum(tiles)
    lo = (total + NCORES - 1) // NCORES
    rng = random.Random(0)
    for tot in range(lo, lo + 13):
        for R in range(4, 11):
            if R > tot:
                break
            for _ in range(3000):
                cuts = sorted(rng.sample(range(1, tot), R - 1)) if R > 1 else []
                s = [b - a for a, b in zip([0] + cuts, cuts + [tot])]
                if max(s) > 32:
                    continue
                s.sort(reverse=True)
                pad, plan = _greedy_assign(s, tiles)
                if pad is not None:
                    # descending size order: big ranges stream while the
                    # pipeline is deep, small ranges make a cheap tail
                    pairs = sorted(zip(s, plan), key=lambda x: -x[0])
                    return [p[0] for p in pairs], [p[1] for p in pairs]
    # fallback: ever-larger uniform capacity until the greedy fits
    for tot in range(lo + 13, lo + 200):
        s = []
        left = tot
        while left > 0:
            s.append(min(16, left))
            left -= s[-1]
        pad, plan = _greedy_assign(s, tiles)
        if pad is not None:
            pairs = sorted(zip(s, plan), key=lambda x: -x[0])
            return [p[0] for p in pairs], [p[1] for p in pairs]
    raise AssertionError("range planner failed")


def _blocks(rs):
    """[(ri, g, tile_off_in_range, bt)], with a small lead block to prime."""
    out = []
    for ri, n in enumerate(rs):
        for g in range(2):
            bo = 0
            if ri == 0 and g == 0 and n > 2:
                out.append((ri, g, 0, 2))
                bo = 2
            while bo < n:
                bt = min(BT, n - bo)
                out.append((ri, g, bo, bt))
                bo += bt
    return out


def _quantize(x):
    absmax = np.abs(x).max(axis=-1)
    scale = np.where(absmax > 0.0, absmax / 127.0, 1.0).astype(np.float32)
    xq = np.clip(np.round(x / scale[..., None]), -127.0, 127.0).astype(np.int8)
    return xq, scale


def _pack_inputs(inputs, rs, plan, blocks):
    q = inputs["q"].reshape(B, NUM_HEADS, D).astype(np.float32)
    k = inputs["k"].reshape(B, KVH, D).astype(np.float32)
    v = inputs["v"].reshape(B, KVH, D).astype(np.float32)
    kc = np.ascontiguousarray(
        inputs["k_cache_q"].reshape(-1, KVH, D).astype(np.int8))
    vc = np.ascontiguousarray(
        inputs["v_cache_q"].reshape(-1, KVH, D).astype(np.int8))
    ks = np.ascontiguousarray(inputs["k_scale"].reshape(-1, KVH)).astype(np.float32)
    vs = np.ascontiguousarray(inputs["v_scale"].reshape(-1, KVH)).astype(np.float32)
    bt_tab = inputs["block_tables"]
    ctx = inputs["context_lens"]
    sm = inputs["slot_mapping"]

    # store_kvcache_int8: quantize the new token and scatter into the cache
    kq, ksn = _quantize(k)
    vq, vsn = _quantize(v)
    kc = kc.copy(); vc = vc.copy(); ks = ks.copy(); vs = vs.copy()
    kc[sm] = kq; vc[sm] = vq; ks[sm] = ksn; vs[sm] = vsn

    R = len(rs)
    RT = sum(rs)
    offs = np.concatenate([[0], np.cumsum(rs)])
    KSZ = sum(bt * P * 4 * D for (_, _, _, bt) in blocks)     # int8 elems
    VSZ = sum(bt * 4 * P * DV for (_, _, _, bt) in blocks)

    # gather + zero-pad each sequence once, globally
    kg_all = {}; vg_all = {}; ksg_all = {}; vsg_all = {}
    for b in range(B):
        nt = int(math.ceil(int(ctx[b]) / P)) * P
        cl = int(ctx[b])
        flat = (bt_tab[b][:, None] * BLOCK_SIZE
                + np.arange(BLOCK_SIZE, dtype=np.int64)[None, :]).reshape(-1)[:nt]
        valid = (np.arange(nt) < cl)
        kg_all[b] = kc[flat] * valid[:, None, None]          # [nt, KVH, D]
        vg = vc[flat] * valid[:, None, None]
        n = nt // P
        vgm = np.zeros((n, P, KVH, DV), dtype=np.int8)
        vgm[:, :, :, :D] = vg.reshape(n, P, KVH, D)
        vgm[:, :, :, D] = valid.reshape(n, P)[:, :, None]
        vg_all[b] = vgm
        ksg_all[b] = (ks[flat] * SCALE) * valid[:, None]     # [nt, KVH]
        vsg_all[b] = vs[flat] * valid[:, None]

    in_maps = []
    for c in range(NCORES):
        kt_c = np.zeros((P, KSZ // P), dtype=np.int8)   # [d, flat]
        vp_c = np.zeros((P, VSZ // P), dtype=np.int8)   # [tok%128, flat]
        scb_c = np.zeros((P, 2, RT, 8), dtype=np.float32)
        qt_c = np.zeros((P, R * 32), dtype=BF16)
        # stage per-range gathered windows (padded to rs[ri] tiles)
        kw = {}; vw = {}
        for ri in range(R):
            n = rs[ri]
            o = int(offs[ri])
            w = plan[ri][c]
            kwin = np.zeros((n * P, KVH, D), dtype=np.int8)
            vwin = np.zeros((n, P, KVH, DV), dtype=np.int8)
            if w is not None:
                b, w0, take = w
                kwin[: take * P] = kg_all[b][w0 * P: (w0 + take) * P]
                vwin[: take] = vg_all[b][w0: w0 + take]
                ksgw = np.zeros((n * P, KVH), dtype=np.float32)
                vsgw = np.full((n * P, KVH), -1e30, dtype=np.float32)
                ksgw[: take * P] = ksg_all[b][w0 * P: (w0 + take) * P]
                with np.errstate(divide="ignore"):
                    vsgw[: take * P] = np.where(
                        vsg_all[b][w0 * P: (w0 + take) * P] > 0.0,
                        np.log(np.maximum(
                            vsg_all[b][w0 * P: (w0 + take) * P], 1e-38)),
                        -1e30)
                scb_c[:, :, o: o + n, 0:4] = (
                    ksgw.reshape(n, P, 2, 4).transpose(1, 2, 0, 3))
                scb_c[:, :, o: o + n, 4:8] = (
                    vsgw.reshape(n, P, 2, 4).transpose(1, 2, 0, 3))
                qt_c[:, ri * 32: (ri + 1) * 32] = q[b].transpose(1, 0).astype(BF16)
            kw[ri] = kwin
            vw[ri] = vwin
        ko = vo = 0
        for (ri, g, bo, bt) in blocks:
            t0, t1 = bo * P, (bo + bt) * P
            kb = kw[ri][t0:t1, 4 * g: 4 * g + 4, :].transpose(2, 1, 0)  # [D,4,btP]
            ksz = 4 * bt * P
            kt_c[:, ko: ko + ksz] = kb.reshape(D, ksz)
            ko += ksz
            vb = vw[ri][bo: bo + bt, :, 4 * g: 4 * g + 4, :].transpose(1, 2, 0, 3)
            vsz = 4 * bt * DV
            vp_c[:, vo: vo + vsz] = vb.reshape(P, vsz)
            vo += vsz
        in_maps.append(dict(kt=kt_c, vp=vp_c, scb=scb_c, qt=qt_c))
    return in_maps


# ---------------------------------------------------------------------------
# device program
# ---------------------------------------------------------------------------

def _build_program(rs):
    blocks = _blocks(rs)
    R = len(rs)
    RT = sum(rs)
    offs = [0]
    for n in rs:
        offs.append(offs[-1] + n)
    KSZ = sum(bt * P * 4 * D for (_, _, _, bt) in blocks)
    VSZ = sum(bt * 4 * P * DV for (_, _, _, bt) in blocks)
    f32 = mybir.dt.float32
    bf16 = mybir.dt.bfloat16
    i8 = mybir.dt.int8
    EXP = mybir.ActivationFunctionType.Exp

    nc = bacc.Bacc("TRN2", target_bir_lowering=False, debug=False,
                   num_devices=NCORES)

    kt_d = nc.dram_tensor("kt", [P, KSZ // P], i8, kind="ExternalInput").ap()
    vp_d = nc.dram_tensor("vp", [P, VSZ // P], i8, kind="ExternalInput").ap()
    scb_d = nc.dram_tensor("scb", [P, 2, RT, 8], f32, kind="ExternalInput").ap()
    qt_d = nc.dram_tensor("qt", [P, R * 32], bf16, kind="ExternalInput").ap()
    out_d = nc.dram_tensor("out", [R, 2, P, DV], f32,
                           kind="ExternalOutput").ap()

    # annotate blocks with flat offsets + group first/last flags
    NB = len(blocks)
    binfo = []
    ko = vo = 0
    nblk = {}
    for (ri, g, _, _) in blocks:
        nblk[(ri, g)] = nblk.get((ri, g), 0) + 1
    seen = {}
    for (ri, g, bo, bt) in blocks:
        seen[(ri, g)] = seen.get((ri, g), 0) + 1
        binfo.append(dict(ri=ri, g=g, bo=bo, bt=bt, ko=ko, vo=vo,
                          first=seen[(ri, g)] == 1,
                          last=seen[(ri, g)] == nblk[(ri, g)]))
        ko += 4 * bt * P
        vo += 4 * bt * DV

    with tile.TileContext(nc) as tc, ExitStack() as ctx:
        const = ctx.enter_context(tc.tile_pool(name="const", bufs=1))
        kt_raw = ctx.enter_context(tc.tile_pool(name="ktr", bufs=6))
        v_raw = ctx.enter_context(tc.tile_pool(name="vpr", bufs=6))
        kt_pool = ctx.enter_context(tc.tile_pool(name="ktp", bufs=3))
        v_pool = ctx.enter_context(tc.tile_pool(name="vpp", bufs=4))
        sc_pool = ctx.enter_context(tc.tile_pool(name="scp", bufs=4))
        work = ctx.enter_context(tc.tile_pool(name="wrk", bufs=3))
        o_pool = ctx.enter_context(tc.tile_pool(name="osb", bufs=2))
        ps_qk = ctx.enter_context(tc.tile_pool(name="psqk", bufs=4, space="PSUM"))
        ps_pv = ctx.enter_context(tc.tile_pool(name="pspv", bufs=3, space="PSUM"))

        qt = const.tile([P, R * 32], bf16)
        nc.sync.dma_start(qt, qt_d)

        st = [dict() for _ in range(NB)]   # per-block live tiles
        grp = {}                           # (ri, g) -> dict(scb=, pv=)

        def dma_stage(t):
            b = binfo[t]
            ri, g, bt = b["ri"], b["g"], b["bt"]
            if b["first"]:
                n = rs[ri]
                o = offs[ri]
                scb = sc_pool.tile([P, n, 8], f32, tag="scb")
                nc.sync.dma_start(scb, scb_d[:, g, o: o + n, :])
                grp[(ri, g)] = dict(scb=scb)
            ksz = 4 * bt * P
            kcr = kt_raw.tile([P, 4, bt, P], i8, tag="ktr")
            nc.sync.dma_start(
                kcr, kt_d[:, b["ko"]: b["ko"] + ksz].rearrange(
                    "d (j i t) -> d j i t", j=4, i=bt))
            vsz3 = 3 * bt * DV
            vsz1 = bt * DV
            vcra = v_raw.tile([P, 3, bt, DV], i8, tag="vpra")
            nc.sync.dma_start(
                vcra, vp_d[:, b["vo"]: b["vo"] + vsz3].rearrange(
                    "p (j i c) -> p j i c", j=3, i=bt))
            vcrb = v_raw.tile([P, 1, bt, DV], i8, tag="vprb")
            nc.sync.dma_start(
                vcrb, vp_d[:, b["vo"] + vsz3: b["vo"] + vsz3 + vsz1].rearrange(
                    "p (j i c) -> p j i c", j=1, i=bt))
            st[t].update(kcr=kcr, vcra=vcra, vcrb=vcrb)

        def cast_qk_stage(t):
            # casts (full-tile: slicing breaks the DVE fast path), then QK
            b = binfo[t]
            ri, g, bt = b["ri"], b["g"], b["bt"]
            kc = kt_pool.tile([P, 4, bt, P], bf16, tag="kt")
            nc.vector.tensor_copy(kc, st[t]["kcr"])
            vca = v_pool.tile([P, 3, bt, DV], bf16, tag="vta")
            nc.scalar.activation(vca, st[t]["vcra"],
                                 mybir.ActivationFunctionType.Copy)
            vcb = v_pool.tile([P, 1, bt, DV], bf16, tag="vtb")
            nc.vector.tensor_copy(vcb, st[t]["vcrb"])
            qk = ps_qk.tile([P, bt, 4, 8], f32, tag="qk")
            for i in range(bt):
                for j2 in range(4):
                    qcol = ri * 32 + (4 * g + j2) * 4
                    nc.tensor.matmul(
                        qk[:, i, j2, 0:4],
                        lhsT=kc[:, j2, i, :],
                        rhs=qt[:, qcol: qcol + 4],
                        start=True, stop=True, skip_group_check=True)
            st[t].update(vca=vca, vcb=vcb, qk=qk)
            del st[t]["kcr"], st[t]["vcra"], st[t]["vcrb"]

        def tail_stage(t):
            # s1 = qk*ksb in place; s2 = s1 + ln(vsb); one exp -> [e | ev];
            # PV+Z matmuls accumulate per (range, group)
            b = binfo[t]
            ri, g, bo, bt = b["ri"], b["g"], b["bo"], b["bt"]
            scb = grp[(ri, g)]["scb"]
            qk = st[t]["qk"]
            nc.vector.tensor_mul(
                qk[:, :, :, 0:4], qk[:, :, :, 0:4],
                scb[:, bo: bo + bt, 0:4].unsqueeze(3).to_broadcast(
                    [P, bt, 4, 4]))
            nc.vector.tensor_add(
                qk[:, :, :, 4:8], qk[:, :, :, 0:4],
                scb[:, bo: bo + bt, 4:8].unsqueeze(3).to_broadcast(
                    [P, bt, 4, 4]))
            ew = work.tile([P, bt, 4, 8], bf16, tag="ew")
            nc.scalar.activation(ew, qk, EXP)
            if b["first"]:
                # no memset: every row the host reads is matmul-written;
                # stale data in unused partitions is copied out and ignored
                pv = ps_pv.tile([P, DV], f32, tag="pv")
                grp[(ri, g)]["pv"] = pv
            pv = grp[(ri, g)]["pv"]
            vca, vcb = st[t]["vca"], st[t]["vcb"]
            for i in range(bt):
                for j2 in range(4):
                    rhs = vca[:, j2, i, :] if j2 < 3 else vcb[:, 0, i, :]
                    nc.tensor.matmul(
                        pv[32 * j2: 32 * j2 + 8, :],
                        lhsT=ew[:, i, j2, :],
                        rhs=rhs,
                        start=(b["first"] and i == 0),
                        stop=(b["last"] and i == bt - 1),
                        tile_position=(0, 32 * j2),
                        skip_group_check=True)
            if b["last"]:
                osb = o_pool.tile([P, DV], f32, tag="osb")
                nc.vector.tensor_copy(osb, pv)
                nc.sync.dma_start(out_d[ri, g], osb)
            st[t].clear()

        LOOK = 3
        for u in range(NB + 2):
            if u == 0:
                for w in range(min(LOOK + 1, NB)):
                    dma_stage(w)
            elif u + LOOK < NB:
                dma_stage(u + LOOK)
            if u < NB:
                cast_qk_stage(u)
            if u >= 2:
                tail_stage(u - 2)

    nc.compile()
    return nc


_PROGRAM_CACHE = {}
_PLAN_CACHE = {}


def _get_program(rs):
    key = tuple(rs)
    if key not in _PROGRAM_CACHE:
        _PROGRAM_CACHE[key] = _build_program(rs)
    return _PROGRAM_CACHE[key]


# ---------------------------------------------------------------------------
# entry point
# ---------------------------------------------------------------------------

def kernel(q, k, v, k_cache_q, v_cache_q, k_scale, v_scale,
           block_tables, context_lens, slot_mapping, _trace=False):
    inputs = dict(q=np.asarray(q), k=np.asarray(k), v=np.asarray(v),
                  k_cache_q=np.asarray(k_cache_q),
                  v_cache_q=np.asarray(v_cache_q),
                  k_scale=np.asarray(k_scale), v_scale=np.asarray(v_scale),
                  block_tables=np.asarray(block_tables),
                  context_lens=np.asarray(context_lens),
                  slot_mapping=np.asarray(slot_mapping))
    ctx_key = inputs["context_lens"].tobytes()
    if ctx_key not in _PLAN_CACHE:
        _PLAN_CACHE[ctx_key] = _plan(inputs["context_lens"])
    rs, plan = _PLAN_CACHE[ctx_key]
    blocks = _blocks(rs)
    in_maps = _pack_inputs(inputs, rs, plan, blocks)
    nc = _get_program(rs)
    res = run_bass_kernel_spmd(nc, in_maps, core_ids=list(range(NCORES)),
                               trace=_trace)

    num = np.zeros((B, NUM_HEADS, D), dtype=np.float64)
    den = np.zeros((B, NUM_HEADS), dtype=np.float64)
    R = len(rs)
    for c in range(NCORES):
        oc = res.results[c]["out"]  # [R, 2, P, DV] f32
        for ri in range(R):
            w = plan[ri][c]
            if w is None:
                continue
            b = w[0]
            for g in range(2):
                for j2 in range(4):
                    j = 4 * g + j2
                    den[b, 4 * j: 4 * j + 4] += oc[ri, g, 32 * j2: 32 * j2 + 4, D]
                    num[b, 4 * j: 4 * j + 4] += oc[ri, g,
                                                   32 * j2 + 4: 32 * j2 + 8, :D]
    out = (num / den[:, :, None]).astype(np.float32).reshape(B, NUM_HEADS * D)
    if _trace:
        return out, res
    return out
